# revision 12
# baseline (speedup 1.0000x reference)
"""Trainium2 Bass kernel for the DEER-MLP spiking network.

Network: x(4,32,196,384) -> FC1(384->1536) -> BatchNorm -> LIF(T=4) ->
FC2(1536->384) -> BatchNorm -> LIF -> spikes(4,32,196,384).

Math note: the reference solves the LIF recurrence with 10 DEER Newton
iterations over T=4 steps. Newton on a length-T triangular system is exact
after T iterations, so the converged result equals the plain sequential
recurrence; we compute that directly (4 elementwise steps).

Distribution: data-parallel over the flattened B*N batch across 8 cores
(784 lanes/core). BatchNorm statistics are the only cross-core coupling:
two tiny AllReduces ([128,24] and [128,6] fp32).

Precision: both matmuls run as multi-pass fp16 with operands split into
hi/lo fp16 limbs (split on host; the PE honors fp16 subnormals). fp16
products accumulate exactly into fp32 PSUM, so FC1 = x_hi@w_hi +
x_lo@w_hi + x_hi@w_lo reproduces fp32 to ~2^-22, and FC2's spikes are
exactly 0/1 in fp16 so two passes (w_hi + w_lo) are ~2^-22 as well.

Orchestration (v2):
  - x limb DMA is chunked so FC1 starts after ~1/7 of the load.
  - A dummy AllReduce at kernel start absorbs inter-core launch skew so
    the BN1 stats AllReduce waits only fabric latency.
  - BN affines run on the Scalar engine (per-partition scale/bias
    activation); spike thresholds run on GpSimd; DVE keeps only the
    sequential h/v chain, so LIF no longer stalls FC2.
  - FC2 m-chunks are (512, 272) so LDWEIGHTS hides under the matmul.
  - The final (channel-partition -> row-major) transpose runs on the PE
    (idle in the tail) instead of 75 serialized sync-queue XBAR DMAs.
"""

import numpy as np

import concourse.bass as bass
import concourse.mybir as mybir
import concourse.tile as tile
from concourse import bacc
from concourse.bass_utils import run_bass_kernel_spmd
F32 = mybir.dt.float32
F16 = mybir.dt.float16
AF = mybir.ActivationFunctionType
OP = mybir.AluOpType
AX = mybir.AxisListType

T, B, NN, C, H = 4, 32, 196, 384, 1536
NCORES = 8
BLOC = B // NCORES            # 4 batches per core
MLOC = BLOC * NN              # 784 lanes per core
R = T * MLOC                  # 3136 flattened (t, m) rows per core
NTOT = T * B * NN             # 25088 batchnorm samples per channel
KC = C // 128                 # 3 c-tiles
KH = H // 128                 # 12 h-tiles
EPS = 1e-5
P = 128

A_CHUNKS = [(i * 448, 448) for i in range(R // 448)]   # 7 uniform chunks
B_CHUNKS = [(0, 392), (392, 392)]   # equal counts: bn_aggr combine assumes them


def _lif(nc, pool, drive, s_out, mlen, tag, spike=("plain",)):
    """Sequential LIF over T steps.

    drive: [128, T, >=mlen] fp32 (already 0.5*BN(y)); s_out: [128, T, >=mlen]
    spike output. h_t = 0.5*v_{t-1} + drive_t; v = h*(h<1).
    h/v chain on DVE. Spike encodings (all exact in fp16):
      ("plain",)       {0,1}    DVE is_ge      (kernel output)
      ("dve",)         {-.5,.5} DVE is_ge-0.5  (= s-0.5; W2 correction in bias)
      ("sign", negone) {-1,1}   Scalar Sign    (host halves that tile's W2)
    """
    def emit_spike(src, t):
        if spike[0] == "sign":
            nc.scalar.activation(s_out[:, t, :mlen], src, AF.Sign,
                                 bias=spike[1][:, 0:1], scale=1.0)
        elif spike[0] == "dve":
            nc.vector.tensor_scalar(s_out[:, t, :mlen], src, 1.0, 0.5,
                                    OP.is_ge, OP.subtract)
        else:
            nc.vector.tensor_scalar(s_out[:, t, :mlen], src, 1.0, None,
                                    OP.is_ge)

    v = pool.tile([P, mlen], F32, tag=f"{tag}_v", name=f"{tag}_v")
    nc.vector.scalar_tensor_tensor(
        v[:], drive[:, 0, :mlen], 1.0, drive[:, 0, :mlen], OP.is_lt, OP.mult
    )
    emit_spike(drive[:, 0, :mlen], 0)
    for t in range(1, T):
        h = pool.tile([P, mlen], F32, tag=f"{tag}_h", name=f"{tag}_h")
        nc.vector.scalar_tensor_tensor(
            h[:], v[:], 0.5, drive[:, t, :mlen], OP.mult, OP.add
        )
        if t < T - 1:
            v = pool.tile([P, mlen], F32, tag=f"{tag}_v", name=f"{tag}_v")
            nc.vector.scalar_tensor_tensor(v[:], h[:], 1.0, h[:], OP.is_lt, OP.mult)
        emit_spike(h[:], t)


def _bn_coeffs(nc, pool, stg, gt, bet, k, tag):
    """From allreduced [128, 2k] (sum || sumsq) compute the fused affine
    drive = y*dsc + dsh  ==  0.5 * ((y - mean) * rsqrt(var+eps) * g + be)."""
    mean = pool.tile([P, k], F32, tag=f"{tag}_mean", name=f"{tag}_mean")
    nc.vector.tensor_scalar(mean[:], stg[:, 0:k], 1.0 / NTOT, None, OP.mult)
    var = pool.tile([P, k], F32, tag=f"{tag}_var", name=f"{tag}_var")
    nc.vector.tensor_scalar(var[:], stg[:, k : 2 * k], 1.0 / NTOT, None, OP.mult)
    msq = pool.tile([P, k], F32, tag=f"{tag}_msq", name=f"{tag}_msq")
    nc.vector.tensor_tensor(msq[:], mean[:], mean[:], OP.mult)
    nc.vector.tensor_tensor(var[:], var[:], msq[:], OP.subtract)
    nc.vector.tensor_scalar(var[:], var[:], EPS, None, OP.add)
    std = pool.tile([P, k], F32, tag=f"{tag}_std", name=f"{tag}_std")
    nc.scalar.activation(std[:], var[:], AF.Sqrt, bias=0.0, scale=1.0)
    rstd = pool.tile([P, k], F32, tag=f"{tag}_rstd", name=f"{tag}_rstd")
    nc.vector.reciprocal(rstd[:], std[:])
    dsc = pool.tile([P, k], F32, tag=f"{tag}_dsc", name=f"{tag}_dsc")
    nc.vector.tensor_tensor(dsc[:], rstd[:], gt[:], OP.mult)
    dsh = pool.tile([P, k], F32, tag=f"{tag}_dsh", name=f"{tag}_dsh")
    nc.vector.tensor_tensor(dsh[:], mean[:], dsc[:], OP.mult)
    nc.vector.tensor_tensor(dsh[:], bet[:], dsh[:], OP.subtract)
    nc.vector.tensor_scalar(dsc[:], dsc[:], 0.5, None, OP.mult)
    nc.vector.tensor_scalar(dsh[:], dsh[:], 0.5, None, OP.mult)
    return dsc, dsh


def _build():
    nc = bacc.Bacc("TRN2", target_bir_lowering=False, debug=False,
                   num_devices=NCORES)

    xh_d = nc.dram_tensor("xthi", [KC, P, R], F16, kind="ExternalInput")
    xl_d = nc.dram_tensor("xtlo", [KC, P, R], F16, kind="ExternalInput")
    w1h_d = nc.dram_tensor("w1thi", [KC, P, H], F16, kind="ExternalInput")
    w1l_d = nc.dram_tensor("w1tlo", [KC, P, H], F16, kind="ExternalInput")
    w2h_d = nc.dram_tensor("w2thi", [KH, P, C], F16, kind="ExternalInput")
    w2l_d = nc.dram_tensor("w2tlo", [KH, P, C], F16, kind="ExternalInput")
    b1_d = nc.dram_tensor("b1", [H], F32, kind="ExternalInput")
    g1_d = nc.dram_tensor("g1", [H], F32, kind="ExternalInput")
    be1_d = nc.dram_tensor("be1", [H], F32, kind="ExternalInput")
    b2_d = nc.dram_tensor("b2", [C], F32, kind="ExternalInput")
    g2_d = nc.dram_tensor("g2", [C], F32, kind="ExternalInput")
    be2_d = nc.dram_tensor("be2", [C], F32, kind="ExternalInput")
    idn_d = nc.dram_tensor("idn", [P, P], F16, kind="ExternalInput")
    out_d = nc.dram_tensor("out", [R, C], F32, kind="ExternalOutput")

    groups = [list(range(NCORES))]

    with tile.TileContext(nc) as tc:
        with (
            tc.tile_pool(name="const", bufs=1) as const,
            tc.tile_pool(name="dram", bufs=1, space="DRAM") as dram,
        ):
            # --- skew absorber: tiny AllReduce on the gpsimd queue -----
            # Launch skew between the 8 NEFFs otherwise lands in the BN1
            # stats AllReduce wait. This dummy completes at
            # max(core arrival) + fabric latency, hidden under FC1.
            dmy = const.tile([1, 4], F32)
            nc.gpsimd.memset(dmy[:], 0.0)
            dmy_in = dram.tile([1, 4], F32)
            dmy_out = dram.tile([1, 4], F32)
            nc.gpsimd.dma_start(dmy_in[:], dmy[:])
            nc.gpsimd.collective_compute(
                "AllReduce", OP.add, replica_groups=groups,
                ins=[dmy_in.opt()], outs=[dmy_out.opt()],
            )
            nc.gpsimd.dma_start(dmy[:], dmy_out[:])

            def colvec(dst_k, src):
                t_ = const.tile([P, dst_k], F32, name=f"cv_{src.name}",
                                tag=f"cv_{src.name}")
                nc.scalar.dma_start(
                    t_[:], src.ap().rearrange("(a p) -> p a", p=P)
                )
                return t_

            b1t, g1t, be1t = (colvec(KH, d) for d in (b1_d, g1_d, be1_d))
            b2t, g2t, be2t = (colvec(KC, d) for d in (b2_d, g2_d, be2_d))

            idn = const.tile([P, P], F16)
            nc.scalar.dma_start(idn[:], idn_d.ap())
            w2h = const.tile([P, KH, C], F16)
            nc.scalar.dma_start(w2h[:], w2h_d.ap().rearrange("k p c -> p k c"))
            w2l = const.tile([P, KH, C], F16)
            nc.scalar.dma_start(w2l[:], w2l_d.ap().rearrange("k p c -> p k c"))

            # --- phase A: FC1 (3-pass fp16) + BN1 partial stats ---------
            y1s = dram.tile([KH, P, R], F32)
            bst1 = const.tile([P, KH, len(A_CHUNKS), 6], F32)
            # gpsimd queue is reserved for collectives + stats DMAs: a
            # load queued behind an in-flight collective would stall.
            wr_q = [nc.sync, nc.sync]
            with (
                tc.tile_pool(name="pax", bufs=1) as pax,
                tc.tile_pool(name="pa", bufs=6) as pa,
                tc.tile_pool(name="ps_mm", bufs=8, space="PSUM") as ps_mm,
            ):
                w1h = pax.tile([P, KC, H], F16)
                nc.sync.dma_start(w1h[:], w1h_d.ap().rearrange("k p h -> p k h"))
                w1l = pax.tile([P, KC, H], F16)
                xh = pax.tile([P, KC, R], F16)
                xl = pax.tile([P, KC, R], F16)
                xh_src = xh_d.ap().rearrange("k p r -> p k r")
                xl_src = xl_d.ap().rearrange("k p r -> p k r")
                for ci, (r0, rlen) in enumerate(A_CHUNKS):
                    nc.sync.dma_start(xh[:, :, r0 : r0 + rlen],
                                      xh_src[:, :, r0 : r0 + rlen])
                    nc.sync.dma_start(xl[:, :, r0 : r0 + rlen],
                                      xl_src[:, :, r0 : r0 + rlen])
                    if ci == 0:
                        nc.sync.dma_start(
                            w1l[:], w1l_d.ap().rearrange("k p h -> p k h"))

                for ci, (r0, rlen) in enumerate(A_CHUNKS):
                    for a in range(KH):
                        ps = ps_mm.tile([P, 512], F32, tag="mm")
                        idx = 0
                        for wt, xt in ((w1h, xh), (w1l, xh), (w1h, xl)):
                            for k in range(KC):
                                nc.tensor.matmul(
                                    ps[:, :rlen],
                                    wt[:, k, a * P : (a + 1) * P],
                                    xt[:, k, r0 : r0 + rlen],
                                    start=(idx == 0),
                                    stop=(idx == 8),
                                )
                                idx += 1
                        y1sb = pa.tile([P, 512], F32, tag="y1sb")
                        nc.scalar.activation(
                            y1sb[:, :rlen], ps[:, :rlen], AF.Identity,
                            bias=b1t[:, a : a + 1], scale=1.0,
                        )
                        nc.vector.bn_stats(bst1[:, a, ci, :], y1sb[:, :rlen])
                        wr_q[(ci * KH + a) % 2].dma_start(
                            y1s[a, :, r0 : r0 + rlen], y1sb[:, :rlen]
                        )

            # --- BN1 stat allreduce -------------------------------------
            st1 = const.tile([P, 2 * KH], F32)
            mv1 = const.tile([P, KH, 2], F32)
            for a in range(KH):
                nc.vector.bn_aggr(mv1[:, a, :], bst1[:, a, :, :])
            # sum = R*mean ; sumsq = R*(var + mean^2)
            nc.vector.tensor_scalar(st1[:, 0:KH], mv1[:, :, 0], float(R),
                                    None, OP.mult)
            msq1 = const.tile([P, KH], F32)
            nc.vector.tensor_tensor(msq1[:], mv1[:, :, 0], mv1[:, :, 0],
                                    OP.mult)
            nc.vector.tensor_tensor(msq1[:], mv1[:, :, 1], msq1[:], OP.add)
            nc.vector.tensor_scalar(st1[:, KH : 2 * KH], msq1[:], float(R),
                                    None, OP.mult)
            st1_in = dram.tile([P, 2 * KH], F32)
            st1_gat = dram.tile([NCORES * P, 2 * KH], F32)
            nc.gpsimd.dma_start(st1_in[:], st1[:])
            nc.gpsimd.collective_compute(
                "AllGather", OP.bypass, replica_groups=groups,
                ins=[st1_in.opt()], outs=[st1_gat.opt()],
            )
            stgg1 = const.tile([P, NCORES, 2 * KH], F32)
            nc.gpsimd.dma_start(
                stgg1[:], st1_gat[:].rearrange("(g p) c -> p g c", p=P)
            )
            stg1 = const.tile([P, 2 * KH], F32)
            nc.vector.tensor_tensor(stg1[:], stgg1[:, 0, :], stgg1[:, 1, :],
                                    OP.add)
            for g in range(2, NCORES):
                nc.vector.tensor_tensor(stg1[:], stg1[:], stgg1[:, g, :],
                                        OP.add)
            dsc1, dsh1 = _bn_coeffs(nc, const, stg1, g1t, be1t, KH, "bn1")
            negone = const.tile([P, 1], F32)
            nc.vector.memset(negone[:], -1.0)

            # --- phase B: BN1 + LIF1 + FC2 (2-pass fp16) + BN2 stats ----
            y2r = [const.tile([P, T, MLOC], F32, tag=f"y2r{ct}",
                              name=f"y2r{ct}")
                   for ct in range(KC)]
            nb2 = len(B_CHUNKS) * T
            bst2 = const.tile([P, KC, nb2, 6], F32)
            rd_q = [nc.sync, nc.sync]
            with (
                tc.tile_pool(name="pb", bufs=2) as pb,
                tc.tile_pool(name="pb_s1", bufs=12) as pbs1,
                tc.tile_pool(name="ps_mm2", bufs=4, space="PSUM") as ps_mm2,
            ):
                # Prefetch chunk-0 y1 while the stat allreduce is in
                # flight (the loads depend only on phase-A scratch
                # writes, not on the collective).
                NPRE = 12
                yt_pre = []
                m0p, mlenp = B_CHUNKS[0]
                for a in range(NPRE):
                    yt = pb.tile([P, T, mlenp], F32, tag="yt_s", bufs=5,
                                 name=f"yt_pre{a}")
                    src = y1s[a].rearrange("p (t m) -> p t m", t=T)
                    rd_q[a % 2].dma_start(yt[:], src[:, :, m0p : m0p + mlenp])
                    yt_pre.append(yt)
                for mi, (m0, mlen) in enumerate(B_CHUNKS):
                    s1_tiles = []
                    for a in range(KH):
                        if mi == 0 and a < NPRE:
                            yt = yt_pre[a]
                        else:
                            yt = pb.tile([P, T, mlen], F32, tag="yt_s",
                                         bufs=5, name=f"yt{mi}_{a}")
                            src = y1s[a].rearrange("p (t m) -> p t m", t=T)
                            rd_q[a % 2].dma_start(yt[:],
                                                  src[:, :, m0 : m0 + mlen])
                        nc.scalar.activation(
                            yt[:, :, :], yt[:, :, :], AF.Identity,
                            bias=dsh1[:, a : a + 1],
                            scale=dsc1[:, a : a + 1],
                        )
                        st_ = pbs1.tile([P, T, mlen], F16,
                                        tag=f"s1_{mi}", bufs=12)
                        spk = ("sign", negone) if a % 2 == 0 else ("dve",)
                        _lif(nc, pb, yt, st_, mlen, "lif1", spike=spk)
                        s1_tiles.append(st_)
                    for t in range(T):
                        for ct in range(KC):
                            ps2 = ps_mm2.tile([P, 512], F32, tag="mm2")
                            idx = 0
                            for k in range(KH):
                                for wsp in (w2h, w2l):
                                    nc.tensor.matmul(
                                        ps2[:, :mlen],
                                        wsp[:, k, ct * P : (ct + 1) * P],
                                        s1_tiles[k][:, t, :mlen],
                                        start=(idx == 0),
                                        stop=(idx == 2 * KH - 1),
                                    )
                                    idx += 1
                            nc.scalar.activation(
                                y2r[ct][:, t, m0 : m0 + mlen], ps2[:, :mlen],
                                AF.Identity, bias=b2t[:, ct : ct + 1],
                                scale=1.0,
                            )

                for ct in range(KC):
                    for mi, (m0, mlen) in enumerate(B_CHUNKS):
                        for t in range(T):
                            nc.vector.bn_stats(
                                bst2[:, ct, mi * T + t, :],
                                y2r[ct][:, t, m0 : m0 + mlen],
                            )

            # --- BN2 stat allreduce -------------------------------------
            st2 = const.tile([P, 2 * KC], F32)
            mv2 = const.tile([P, KC, 2], F32)
            for ct in range(KC):
                nc.vector.bn_aggr(mv2[:, ct, :], bst2[:, ct, :, :])
            nc.vector.tensor_scalar(st2[:, 0:KC], mv2[:, :, 0], float(R),
                                    None, OP.mult)
            msq2 = const.tile([P, KC], F32)
            nc.vector.tensor_tensor(msq2[:], mv2[:, :, 0], mv2[:, :, 0],
                                    OP.mult)
            nc.vector.tensor_tensor(msq2[:], mv2[:, :, 1], msq2[:], OP.add)
            nc.vector.tensor_scalar(st2[:, KC : 2 * KC], msq2[:], float(R),
                                    None, OP.mult)
            st2_in = dram.tile([P, 2 * KC], F32)
            st2_gat = dram.tile([NCORES * P, 2 * KC], F32)
            nc.gpsimd.dma_start(st2_in[:], st2[:])
            nc.gpsimd.collective_compute(
                "AllGather", OP.bypass, replica_groups=groups,
                ins=[st2_in.opt()], outs=[st2_gat.opt()],
            )
            stgg2 = const.tile([P, NCORES, 2 * KC], F32)
            nc.gpsimd.dma_start(
                stgg2[:], st2_gat[:].rearrange("(g p) c -> p g c", p=P)
            )
            stg2 = const.tile([P, 2 * KC], F32)
            nc.vector.tensor_tensor(stg2[:], stgg2[:, 0, :], stgg2[:, 1, :],
                                    OP.add)
            for g in range(2, NCORES):
                nc.vector.tensor_tensor(stg2[:], stg2[:], stgg2[:, g, :],
                                        OP.add)
            dsc2, dsh2 = _bn_coeffs(nc, const, stg2, g2t, be2t, KC, "bn2")

            # --- phase C: BN2 (in place) + LIF2 -> fp16 spikes ----------
            # --- phase D: PE transpose + scalar evac + row-major DMA ----
            NRB = (R + P - 1) // P              # 25 row blocks (last 64)
            out_q = [nc.sync, nc.gpsimd, nc.scalar]
            with (
                tc.tile_pool(name="ps2", bufs=1) as ps2p,
                tc.tile_pool(name="pc", bufs=3) as pc,
                tc.tile_pool(name="pd", bufs=6) as pd,
                tc.tile_pool(name="ps_tp", bufs=6, space="PSUM") as ps_tp,
            ):
                s2t = [ps2p.tile([P, R], F16, tag=f"s2t{ct}", name=f"s2t{ct}")
                       for ct in range(KC)]
                ob = [ps2p.tile([P, C], F32, tag=f"ob{rb}", name=f"ob{rb}")
                      for rb in range(NRB)]
                for ct in range(KC):
                    for t in range(T):
                        nc.scalar.activation(
                            y2r[ct][:, t, :], y2r[ct][:, t, :], AF.Identity,
                            bias=dsh2[:, ct : ct + 1],
                            scale=dsc2[:, ct : ct + 1],
                        )
                    s2v = s2t[ct][:, :].rearrange("p (t m) -> p t m", t=T)
                    _lif(nc, pc, y2r[ct], s2v, MLOC, "lif2")

                    for rb in range(NRB):
                        r0 = rb * P
                        rlen = min(P, R - r0)
                        pst = ps_tp.tile([P, P], F16, tag="tp")
                        nc.tensor.transpose(
                            pst[:rlen, :], s2t[ct][:, r0 : r0 + rlen], idn[:]
                        )
                        nc.scalar.activation(
                            ob[rb][:rlen, ct * P : (ct + 1) * P],
                            pst[:rlen, :], AF.Copy,
                        )
                        if ct == KC - 1:
                            out_q[rb % 3].dma_start(
                                out_d[r0 : r0 + rlen, :], ob[rb][:rlen, :]
                            )

    nc.compile()
    return nc


_NC = None
TRACE = False          # set by test harness to capture an NTFF profile
LAST_RESULT = None     # BassKernelResults of the most recent run


def _get_nc():
    global _NC
    if _NC is None:
        _NC = _build()
    return _NC


def _split_f16(a):
    hi = a.astype(np.float16)
    lo = (a - hi.astype(np.float32)).astype(np.float16)
    return np.ascontiguousarray(hi), np.ascontiguousarray(lo)


def _in_maps(x, W1, b1, g1, be1, W2, b2, g2, be2):
    x = np.asarray(x, dtype=np.float32)
    w1t = np.asarray(W1, np.float32).T.reshape(KC, P, H)
    w1thi, w1tlo = _split_f16(w1t)
    w2t = np.asarray(W2, np.float32).T.reshape(KH, P, C)
    w2thi, w2tlo = _split_f16(w2t)
    # Spikes arrive as s-1/2 (DVE tiles, +-0.5) or 2s-1 (Sign tiles, +-1,
    # with those tiles' weights halved below): either way each k-tile
    # contributes s@W - 0.5*colsum(W); add the 0.5*colsum back via b2.
    b2c = np.asarray(b2, np.float32) + 0.5 * (
        w2thi.astype(np.float32) + w2tlo.astype(np.float32)
    ).sum(axis=(0, 1))
    w2thi = w2thi.copy()
    w2tlo = w2tlo.copy()
    for k in range(KH):
        if k % 2 == 0:
            w2thi[k] = (w2thi[k].astype(np.float32) * 0.5).astype(np.float16)
            w2tlo[k] = (w2tlo[k].astype(np.float32) * 0.5).astype(np.float16)
    shared = {
        "w1thi": w1thi, "w1tlo": w1tlo,
        "w2thi": w2thi, "w2tlo": w2tlo,
        "b1": np.asarray(b1, np.float32),
        "g1": np.asarray(g1, np.float32),
        "be1": np.asarray(be1, np.float32),
        "b2": b2c,
        "g2": np.asarray(g2, np.float32),
        "be2": np.asarray(be2, np.float32),
        "idn": np.eye(P, dtype=np.float16),
    }
    in_maps = []
    for i in range(NCORES):
        xt = x[:, i * BLOC : (i + 1) * BLOC].reshape(R, C).T.reshape(KC, P, R)
        xthi, xtlo = _split_f16(xt)
        in_maps.append({"xthi": xthi, "xtlo": xtlo, **shared})
    return in_maps


def kernel(x, W1, b1, g1, be1, W2, b2, g2, be2):
    nc = _get_nc()
    in_maps = _in_maps(x, W1, b1, g1, be1, W2, b2, g2, be2)
    res = run_bass_kernel_spmd(nc, in_maps, core_ids=list(range(NCORES)),
                               trace=TRACE)
    global LAST_RESULT
    LAST_RESULT = res
    out = np.concatenate(
        [res.results[i]["out"].reshape(T, BLOC, NN, C) for i in range(NCORES)],
        axis=1,
    )
    return out
